# revision 46
# baseline (speedup 1.0000x reference)
"""EdgeGuidance Trainium2 kernel.

Pipeline per image [3,544,960] -> [1,136,240]:
  gray = w.RGB  ->  smooth = gauss5x5(reflect)  ->  gx,gy = sobel(zero-pad)
  mag = sqrt(gx^2+gy^2)  ->  4x4 avgpool  ->  sigmoid(5(x-0.2))^2

All linear steps fold into two banded-matrix passes on the PE in fp16
(1 cycle/row; rel err ~2e-3 vs 2e-2 budget):
  gx = A_x @ gray @ Bx^T,   gy = A_y @ gray @ By^T
There is NO explicit gray pass: inputs are cast f32->fp16 during the
SWDGE DMA itself, and the channel weights are folded into three
pre-scaled copies of the phase-A band; phase A accumulates the three
channel matmuls in PSUM.  Phase A uses the rgb slab as the matmul
stationary so its output lands transposed ([w, s]); each of 5 row-blocks
owns a disjoint s-window (rows overlap by 6 so no cross-block PSUM
accumulation is needed).  Phase B contracts over w with the B^T band
stationary.

The cast-during-DMA halves the SBUF-side fabric traffic, so the input
stream sustains ~345 GB/s read-side (~38 us for 13.3 MB); the PE
(~47 us of LDWEIGHTS+MATMUL) is the binding engine, so the schedule is
built around keeping its queue head unblocked: single-buffered psum
slots are copied out in the same order the next chunk's matmuls are
emitted, and each tap's Square runs on ACT while the PE streams the
other tap.  Elementwise tail: DVE does all psum->sbuf fp16 copies, the
fp16 m2 add (2x mode) and the 4x reduce; ACT does Square/Sqrt/sigmoid;
POOL does SWDGE issue + the final squares.  Image 0 loads in W-eighth/
quarter steps (compute starts after ~1 MB), image 1 in halves (its
chunks start right as image 0 drains).

Data parallel over batch: 8 cores x 2 images.
"""

import numpy as np

import concourse.bass as bass
import concourse.tile as tile
from concourse import mybir
from concourse.bass_utils import run_bass_kernel_spmd

F32 = mybir.dt.float32
F16 = mybir.dt.float16
AF = mybir.ActivationFunctionType
ALU = mybir.AluOpType

B_FULL, C, H, W = 16, 3, 544, 960
N_CORES = 8
B_LOC = B_FULL // N_CORES  # images per core
HP, WP = H // 4, W // 4  # 136, 240

BLUR_K, SIGMA = 5, 1.5
W_R, W_G, W_B = 0.2989, 0.587, 0.114

# 5 row-blocks (k multiple of 8 for DMA engine spray), each owning a
# disjoint s-window; rows [s-3, s+4) of every owned s lie inside the block.
GB = [(0, 120), (111, 231), (225, 345), (339, 459), (448, 544)]
SW = [(0, 114), (114, 228), (228, 342), (342, 456), (456, 544)]
# phase-A psum packing: slots 0,1 in a01 [456], slots 2,3 in a23 [456],
# slot 4 in a4 [176], each 1 bank x bufs=1; phase-B taps gx/gy get a
# 2-bank tile each -- 1+1+1 + 2+2 + psP 1 = 8 banks exactly.
N_WC = 8  # w-chunks of 120 outputs each


def _wj(j):
    return max(0, 120 * j - 4), min(W, 120 * j + 124)


# ---------------------------------------------------------------- numpy bands
def _blur1d():
    x = np.arange(BLUR_K, dtype=np.float64) - (BLUR_K - 1) / 2.0
    g = np.exp(-(x**2) / (2.0 * SIGMA**2))
    return g / g.sum()


def _band_reflect(n, taps):
    r = len(taps) // 2
    m = np.zeros((n, n), dtype=np.float64)
    for s in range(n):
        for d in range(-r, r + 1):
            i = s + d
            if i < 0:
                i = -i
            elif i >= n:
                i = 2 * n - 2 - i
            m[s, i] += taps[d + r]
    return m


def _band_zero(n, taps):
    r = len(taps) // 2
    m = np.zeros((n, n), dtype=np.float64)
    for s in range(n):
        for d in range(-r, r + 1):
            i = s + d
            if 0 <= i < n:
                m[s, i] += taps[d + r]
    return m


def build_constants():
    f16 = np.float16
    g1 = _blur1d()
    vb_h = _band_reflect(H, g1)  # vertical blur on H
    hb_w = _band_reflect(W, g1)  # horizontal blur on W
    ax = _band_zero(H, [1.0, 2.0, 1.0]) @ vb_h
    ay = _band_zero(H, [-1.0, 0.0, 1.0]) @ vb_h
    bx = _band_zero(W, [-1.0, 0.0, 1.0]) @ hb_w
    by = _band_zero(W, [1.0, 2.0, 1.0]) @ hb_w

    # phase A: per channel c (scaled by its gray weight), 5 slots packed
    # back-to-back, interleaved (s, t): col c*1088 + off_i + 2u + t
    band_a = np.zeros((128, 3 * 1088), dtype=np.float64)
    for c, wc in enumerate((W_R, W_G, W_B)):
        off = 0
        for (r0, r1), (s0, s1) in zip(GB, SW):
            k, w_ = r1 - r0, s1 - s0
            blk = np.stack(
                [wc * ax[s0:s1, r0:r1], wc * ay[s0:s1, r0:r1]], axis=-1
            )  # [w,k,2]
            band_a[0:k, c * 1088 + off : c * 1088 + off + 2 * w_] = blk.transpose(
                1, 0, 2
            ).reshape(k, 2 * w_)
            off += 2 * w_

    # phase B: per (t, j) block [mj, 120] at cols (t*8+j)*120
    band_b = np.zeros((128, 2 * N_WC * 120), dtype=np.float64)
    for t, m in enumerate((bx, by)):
        for j in range(N_WC):
            w0, w1 = _wj(j)
            blk = m[120 * j : 120 * j + 120, w0:w1].T  # [mj, 120]
            band_b[0 : w1 - w0, (t * N_WC + j) * 120 : (t * N_WC + j + 1) * 120] = blk

    p4 = np.zeros((128, 30), dtype=np.float64)
    for wp in range(120):
        p4[wp, wp // 4] = 1.0 / 16.0
    return (
        band_a.astype(f16),
        band_b.astype(f16),
        p4.astype(f16),
    )


# ------------------------------------------------------------------ bass build
def split_multi_waits(nc):
    """walrus in this container only accepts 1 sync-wait per instruction;
    hoist extra waits onto preceding same-engine NoOps."""
    for fn in nc.m.functions:
        for bb in fn.blocks:
            new_list, changed = [], False
            for ins in bb.instructions:
                si = ins.sync_info
                waits = list(si.on_wait) if si is not None else []
                if len(waits) > 1:
                    changed = True
                    for i, wt in enumerate(waits[:-1]):
                        new_list.append(
                            mybir.InstNoOp(
                                name=f"{ins.name}_ws{i}",
                                engine=ins.engine,
                                bass_nofuse=True,
                                sync_info=mybir.SyncInfo(on_wait=[wt], on_update=[]),
                            )
                        )
                    si.on_wait = [waits[-1]]
                    ins.sync_info = si
                new_list.append(ins)
            if changed:
                bb.instructions = new_list


def build_module():
    nc = bass.Bass("TRN2", target_bir_lowering=False, debug=False)
    x = nc.dram_tensor("x", [B_LOC, C, H, W], F32, kind="ExternalInput").ap()
    ba = nc.dram_tensor("bA", [128, 3 * 1088], F16, kind="ExternalInput").ap()
    bb_ = nc.dram_tensor("bB", [128, 2 * N_WC * 120], F16, kind="ExternalInput").ap()
    p4 = nc.dram_tensor("p4", [128, 30], F16, kind="ExternalInput").ap()
    y = nc.dram_tensor("y", [B_LOC, 1, HP, WP], F32, kind="ExternalOutput").ap()

    with tile.TileContext(nc) as tc:
        with (
            tc.tile_pool(name="const", bufs=1) as cpool,
            tc.tile_pool(name="rgb", bufs=10) as rgbp,
            tc.tile_pool(name="xy", bufs=6) as xyp,
            tc.tile_pool(name="sq", bufs=3) as sqp,
            tc.tile_pool(name="sp", bufs=3) as spp,
            tc.tile_pool(name="outp", bufs=2) as outp,
            tc.tile_pool(name="psA1", bufs=1, space="PSUM") as psA1,
            tc.tile_pool(name="psA2", bufs=1, space="PSUM") as psA2,
            tc.tile_pool(name="psA3", bufs=1, space="PSUM") as psA3,
            tc.tile_pool(name="psBx", bufs=1, space="PSUM") as psBx,
            tc.tile_pool(name="psBy", bufs=1, space="PSUM") as psBy,
            tc.tile_pool(name="psP", bufs=1, space="PSUM") as psP,
            nc.allow_low_precision(reason="fp16 pipeline, tolerance 2e-2"),
        ):
            # ---- constants first on the sync HWDGE ring so phase A can
            # start as soon as the first rgb block lands
            ba_t = cpool.tile([128, 3 * 1088], F16, tag="ba")
            nc.sync.dma_start(ba_t[:], ba[:])
            bb_t = cpool.tile([128, 2 * N_WC * 120], F16, tag="bb")
            nc.sync.dma_start(bb_t[:], bb_[:])
            p4_t = cpool.tile([128, 30], F16, tag="p4")
            nc.sync.dma_start(p4_t[:], p4[:])

            # ---- input loads: SWDGE casts f32->fp16 in flight.
            # image 0 is loaded in W-halves (all blocks' first halves land
            # first, so phase A chunks 0-3 start ~5us earlier); image 1 as
            # whole blocks.  Halves overlap 16 cols for the +-4 w margins.
            def load_block(rgb, b, r0, r1, wl, wr_):
                k = r1 - r0
                nc.gpsimd.dma_start(
                    rgb[0:k, :].rearrange("p (c w) -> p c w", c=3)[:, :, wl:wr_],
                    x[b, :, r0:r1, wl:wr_].rearrange("c p w -> p c w"),
                )

            def emit_loads(b, splits):
                rgbs = [rgbp.tile([128, 3 * W], F16, tag="rgb", name="rgb")
                        for _ in GB]
                for wl, wr_ in splits:
                    for i, (r0, r1) in enumerate(GB):
                        load_block(rgbs[i], b, r0, r1, wl, wr_)
                return rgbs

            # image 0 in W-quarters (phase A starts after 1/4 of the image),
            # image 1 in W-halves (its first chunks start right as image 0's
            # compute drains, instead of waiting for the whole image)
            rgbs_all = {
                0: emit_loads(0, [(0, 128), (116, 244), (236, 484),
                                  (476, 724), (716, W)]),
                1: emit_loads(1, [(0, 488), (472, W)]),
            }

            bias_m1 = cpool.tile([128, 1], F32, tag="bm1")
            nc.gpsimd.memset(bias_m1[:], -1.0)

            stores = []

            for b in range(B_LOC):
                rgb_t = rgbs_all[b]
                pooled = psP.tile([128, 2 * WP], F32, tag="pooled")

                # per-block col offset inside the packed 1088 layout
                AOFF = [0, 228, 456, 684, 912]

                def stage_a(j):
                    """phase A: 5 slots x 3 channel-accumulated matmuls.
                    Emission order 0,1,4,2,3 matches the copy order, so each
                    next-chunk matmul's psum slot is freed by the time the
                    PE queue head reaches it."""
                    w0, w1 = _wj(j)
                    mj = w1 - w0
                    a01 = psA1.tile([128, 456], F32, tag="a01")
                    a23 = psA2.tile([128, 456], F32, tag="a23")
                    a4 = psA3.tile([128, 176], F32, tag="a4")
                    for i in (0, 1, 4, 2, 3):
                        (r0, r1), (s0, s1) = GB[i], SW[i]
                        k = r1 - r0
                        wid = 2 * (s1 - s0)
                        off = AOFF[i]
                        if i < 2:
                            dst = a01[0:mj, off : off + wid]
                        elif i < 4:
                            dst = a23[0:mj, off - 456 : off - 456 + wid]
                        else:
                            dst = a4[0:mj, 0:wid]
                        for c in range(3):
                            nc.tensor.matmul(
                                dst,
                                rgb_t[i][0:k, c * W + w0 : c * W + w1],
                                ba_t[0:k, c * 1088 + off : c * 1088 + off + wid],
                                start=(c == 0),
                                stop=(c == 2),
                            )
                    return a01, a23, a4

                def stage_copy(j, a01, a23, a4):
                    """psum -> sbuf xy fp16 on DVE; single-buffered slots
                    (a4, a01) first so the next chunk's matmuls unblock."""
                    w0, w1 = _wj(j)
                    mj = w1 - w0
                    xy = xyp.tile([128, 1088], F16, tag="xy")
                    nc.vector.tensor_copy(xy[0:mj, 912:1088], a4[0:mj, :])
                    nc.vector.tensor_copy(xy[0:mj, 0:456], a01[0:mj, :])
                    # a23 on ACT: frees the last-emitted phase-A slot in
                    # parallel with DVE's a01/a4 casts
                    nc.scalar.copy(xy[0:mj, 456:912], a23[0:mj, :])
                    return xy

                def stage_b(j, xy):
                    """phase B per tap into its own psum tile; the Square of
                    gx is emitted between the gx and gy matmuls so it runs on
                    ACT while the PE streams gy -- both tap tiles are free by
                    the time the next chunk's B matmuls reach the PE head."""
                    w0, w1 = _wj(j)
                    mj = w1 - w0
                    xyv = xy[0:mj, :].rearrange("p (s two) -> p two s", two=2)
                    sqs = []
                    for t, ps in ((0, psBx), (1, psBy)):
                        g = ps.tile([128, 768], F32, tag=f"g{t}", name=f"g{t}")
                        bT = bb_t[
                            0:mj, (t * N_WC + j) * 120 : (t * N_WC + j + 1) * 120
                        ]
                        nc.tensor.matmul(
                            g[0:120, 224:512], bT, xyv[:, t, 0:288],
                            start=True, stop=True,
                        )
                        nc.tensor.matmul(
                            g[0:120, 512:768], bT, xyv[:, t, 288:H],
                            start=True, stop=True,
                        )
                        sq = sqp.tile([128, H], F16, tag=f"sq{t}", name=f"sq{t}")
                        nc.scalar.activation(
                            sq[0:120, :], g[0:120, 224:768], AF.Square
                        )
                        sqs.append(sq)
                    m2 = sqp.tile([128, H], F16, tag="m2")
                    # image 0: DVE (POOL is still emitting SWDGE loads);
                    # image 1: POOL, so the DVE FIFO reaches the psum-freeing
                    # casts sooner and the PE queue head never waits on them
                    eng = nc.vector if b == 0 else nc.gpsimd
                    eng.tensor_add(
                        m2[0:120, :], sqs[0][0:120, :], sqs[1][0:120, :]
                    )
                    return m2

                def stage_mag2(j, m2):
                    mg = sqp.tile([128, H], F16, tag="mg")
                    nc.scalar.activation(mg[0:120, :], m2[0:120, :], AF.Sqrt)
                    sp = spp.tile([128, HP], F16, tag="sp")
                    nc.vector.tensor_reduce(
                        sp[0:120, :],
                        mg[0:120, :].rearrange("p (g f) -> p g f", f=4),
                        axis=mybir.AxisListType.X,
                        op=ALU.add,
                    )
                    return sp

                def stage_pool(j, sp):
                    nc.tensor.matmul(
                        pooled[0:96, 30 * j : 30 * j + 30],
                        sp[0:120, 0:96],
                        p4_t[0:120, :],
                        start=True,
                        stop=True,
                    )
                    nc.tensor.matmul(
                        pooled[0:40, WP + 30 * j : WP + 30 * j + 30],
                        sp[0:120, 96:HP],
                        p4_t[0:120, :],
                        start=True,
                        stop=True,
                    )

                # software-pipelined emission: PE queue order pool(j-1),
                # A(j+1), B(j); the whole mag chain (Square, add, sqrt,
                # reduce) runs within iteration j so the drain tail is short
                # and the single psB buffer frees before B(j+1) hits the PE
                aout = {0: stage_a(0)}
                m2s, sps = {}, {}
                for j in range(N_WC + 3):
                    if 0 <= j < N_WC:
                        xy = stage_copy(j, *aout.pop(j))
                    if j - 3 in sps:
                        stage_pool(j - 3, sps.pop(j - 3))
                    if j + 1 < N_WC:
                        aout[j + 1] = stage_a(j + 1)
                    if 0 <= j < N_WC:
                        m2s[j] = stage_b(j, xy)
                    if j - 1 in m2s:
                        sps[j - 1] = stage_mag2(j - 1, m2s.pop(j - 1))

                # ---- sigmoid(5x-1)^2 on pooled, store deferred to the end
                sg = outp.tile([128, 2 * WP], F32, tag="sg")
                nc.scalar.activation(
                    sg[0:96, :], pooled[0:96, :], AF.Sigmoid,
                    bias=bias_m1[0:96, :], scale=5.0,
                )
                ot = outp.tile([128, 2 * WP], F32, tag="ot")
                nc.gpsimd.tensor_mul(ot[0:96, :], sg[0:96, :], sg[0:96, :])
                stores.append((b, ot))

            for b, ot in stores:
                nc.sync.dma_start(y[b, 0, 0:96, :], ot[0:96, 0:WP])
                nc.sync.dma_start(y[b, 0, 96:HP, :], ot[0:40, WP : 2 * WP])

    split_multi_waits(nc)
    return nc


_NC = None
_CONSTS = None
TRACE = False
LAST_EXEC_NS = None


def kernel(**inputs):
    global _NC, _CONSTS, LAST_EXEC_NS
    left_rgb = np.ascontiguousarray(np.asarray(inputs["left_rgb"], dtype=np.float32))
    assert left_rgb.shape == (B_FULL, C, H, W)
    if _NC is None:
        _NC = build_module()
        _CONSTS = build_constants()
    band_a, band_b, p4 = _CONSTS
    in_maps = [
        {
            "x": np.ascontiguousarray(left_rgb[i * B_LOC : (i + 1) * B_LOC]),
            "bA": band_a,
            "bB": band_b,
            "p4": p4,
        }
        for i in range(N_CORES)
    ]
    res = run_bass_kernel_spmd(
        _NC, in_maps, core_ids=list(range(N_CORES)), trace=TRACE
    )
    LAST_EXEC_NS = res.exec_time_ns
    out = np.empty((B_FULL, 1, HP, WP), dtype=np.float32)
    for i in range(N_CORES):
        out[i * B_LOC : (i + 1) * B_LOC] = res.results[i]["y"]
    return out


# revision 47
# speedup vs baseline: 1.1423x; 1.1423x over previous
"""EdgeGuidance Trainium2 kernel.

Pipeline per image [3,544,960] -> [1,136,240]:
  gray = w.RGB  ->  smooth = gauss5x5(reflect)  ->  gx,gy = sobel(zero-pad)
  mag = sqrt(gx^2+gy^2)  ->  4x4 avgpool  ->  sigmoid(5(x-0.2))^2

All linear steps fold into two banded-matrix passes on the PE in fp16
(1 cycle/row; rel err ~2e-3 vs 2e-2 budget):
  gx = A_x @ gray @ Bx^T,   gy = A_y @ gray @ By^T
There is NO explicit gray pass: inputs are cast f32->fp16 during the
SWDGE DMA itself, and the channel weights are folded into three
pre-scaled copies of the phase-A band; phase A accumulates the three
channel matmuls in PSUM.  Phase A uses the rgb slab as the matmul
stationary so its output lands transposed ([w, s]); each of 5 row-blocks
owns a disjoint s-window (rows overlap by 6 so no cross-block PSUM
accumulation is needed).  Phase B contracts over w with the B^T band
stationary.

The cast-during-DMA halves the SBUF-side fabric traffic, so the input
stream sustains ~345 GB/s read-side (~38 us for 13.3 MB); the PE
(~47 us of LDWEIGHTS+MATMUL) is the binding engine, so the schedule is
built around keeping its queue head unblocked: single-buffered psum
slots are copied out in the same order the next chunk's matmuls are
emitted, and each tap's Square runs on ACT while the PE streams the
other tap.  Elementwise tail: DVE does all psum->sbuf fp16 copies, the
fp16 m2 add (2x mode) and the 4x reduce; ACT does Square/Sqrt/sigmoid;
POOL does SWDGE issue + the final squares.  Image 0 loads in W-eighth/
quarter steps (compute starts after ~1 MB), image 1 in halves (its
chunks start right as image 0 drains).

Data parallel over batch: 8 cores x 2 images.
"""

import numpy as np

import concourse.bass as bass
import concourse.tile as tile
from concourse import mybir
from concourse.bass_utils import run_bass_kernel_spmd

F32 = mybir.dt.float32
F16 = mybir.dt.float16
AF = mybir.ActivationFunctionType
ALU = mybir.AluOpType

B_FULL, C, H, W = 16, 3, 544, 960
N_CORES = 8
B_LOC = B_FULL // N_CORES  # images per core
HP, WP = H // 4, W // 4  # 136, 240

BLUR_K, SIGMA = 5, 1.5
W_R, W_G, W_B = 0.2989, 0.587, 0.114

# 5 row-blocks (k multiple of 8 for DMA engine spray), each owning a
# disjoint s-window; rows [s-3, s+4) of every owned s lie inside the block.
GB = [(0, 120), (111, 231), (225, 345), (339, 459), (448, 544)]
SW = [(0, 114), (114, 228), (228, 342), (342, 456), (456, 544)]
# phase-A psum packing: slots 0,1 in a01 [456], slots 2,3 in a23 [456],
# slot 4 in a4 [176], each 1 bank x bufs=1; phase-B taps gx/gy get a
# 2-bank tile each -- 1+1+1 + 2+2 + psP 1 = 8 banks exactly.
N_WC = 8  # w-chunks of 120 outputs each


def _wj(j):
    return max(0, 120 * j - 4), min(W, 120 * j + 124)


# ---------------------------------------------------------------- numpy bands
def _blur1d():
    x = np.arange(BLUR_K, dtype=np.float64) - (BLUR_K - 1) / 2.0
    g = np.exp(-(x**2) / (2.0 * SIGMA**2))
    return g / g.sum()


def _band_reflect(n, taps):
    r = len(taps) // 2
    m = np.zeros((n, n), dtype=np.float64)
    for s in range(n):
        for d in range(-r, r + 1):
            i = s + d
            if i < 0:
                i = -i
            elif i >= n:
                i = 2 * n - 2 - i
            m[s, i] += taps[d + r]
    return m


def _band_zero(n, taps):
    r = len(taps) // 2
    m = np.zeros((n, n), dtype=np.float64)
    for s in range(n):
        for d in range(-r, r + 1):
            i = s + d
            if 0 <= i < n:
                m[s, i] += taps[d + r]
    return m


def build_constants():
    f16 = np.float16
    g1 = _blur1d()
    vb_h = _band_reflect(H, g1)  # vertical blur on H
    hb_w = _band_reflect(W, g1)  # horizontal blur on W
    ax = _band_zero(H, [1.0, 2.0, 1.0]) @ vb_h
    ay = _band_zero(H, [-1.0, 0.0, 1.0]) @ vb_h
    bx = _band_zero(W, [-1.0, 0.0, 1.0]) @ hb_w
    by = _band_zero(W, [1.0, 2.0, 1.0]) @ hb_w

    # phase A: per channel c (scaled by its gray weight), 5 slots packed
    # back-to-back, interleaved (s, t): col c*1088 + off_i + 2u + t
    band_a = np.zeros((128, 3 * 1088), dtype=np.float64)
    for c, wc in enumerate((W_R, W_G, W_B)):
        off = 0
        for (r0, r1), (s0, s1) in zip(GB, SW):
            k, w_ = r1 - r0, s1 - s0
            blk = np.stack(
                [wc * ax[s0:s1, r0:r1], wc * ay[s0:s1, r0:r1]], axis=-1
            )  # [w,k,2]
            band_a[0:k, c * 1088 + off : c * 1088 + off + 2 * w_] = blk.transpose(
                1, 0, 2
            ).reshape(k, 2 * w_)
            off += 2 * w_

    # phase B: per (t, j) block [mj, 120] at cols (t*8+j)*120
    band_b = np.zeros((128, 2 * N_WC * 120), dtype=np.float64)
    for t, m in enumerate((bx, by)):
        for j in range(N_WC):
            w0, w1 = _wj(j)
            blk = m[120 * j : 120 * j + 120, w0:w1].T  # [mj, 120]
            band_b[0 : w1 - w0, (t * N_WC + j) * 120 : (t * N_WC + j + 1) * 120] = blk

    p4 = np.zeros((128, 30), dtype=np.float64)
    for wp in range(120):
        p4[wp, wp // 4] = 1.0 / 16.0
    return (
        band_a.astype(f16),
        band_b.astype(f16),
        p4.astype(f16),
    )


# ------------------------------------------------------------------ bass build
def split_multi_waits(nc):
    """walrus in this container only accepts 1 sync-wait per instruction;
    hoist extra waits onto preceding same-engine NoOps."""
    for fn in nc.m.functions:
        for bb in fn.blocks:
            new_list, changed = [], False
            for ins in bb.instructions:
                si = ins.sync_info
                waits = list(si.on_wait) if si is not None else []
                if len(waits) > 1:
                    changed = True
                    for i, wt in enumerate(waits[:-1]):
                        new_list.append(
                            mybir.InstNoOp(
                                name=f"{ins.name}_ws{i}",
                                engine=ins.engine,
                                bass_nofuse=True,
                                sync_info=mybir.SyncInfo(on_wait=[wt], on_update=[]),
                            )
                        )
                    si.on_wait = [waits[-1]]
                    ins.sync_info = si
                new_list.append(ins)
            if changed:
                bb.instructions = new_list


def build_module():
    nc = bass.Bass("TRN2", target_bir_lowering=False, debug=False)
    x = nc.dram_tensor("x", [B_LOC, C, H, W], F32, kind="ExternalInput").ap()
    ba = nc.dram_tensor("bA", [128, 3 * 1088], F16, kind="ExternalInput").ap()
    bb_ = nc.dram_tensor("bB", [128, 2 * N_WC * 120], F16, kind="ExternalInput").ap()
    p4 = nc.dram_tensor("p4", [128, 30], F16, kind="ExternalInput").ap()
    y = nc.dram_tensor("y", [B_LOC, 1, HP, WP], F32, kind="ExternalOutput").ap()

    with tile.TileContext(nc) as tc:
        with (
            tc.tile_pool(name="const", bufs=1) as cpool,
            tc.tile_pool(name="rgb", bufs=10) as rgbp,
            tc.tile_pool(name="xy", bufs=6) as xyp,
            tc.tile_pool(name="sq", bufs=3) as sqp,
            tc.tile_pool(name="sp", bufs=3) as spp,
            tc.tile_pool(name="outp", bufs=2) as outp,
            tc.tile_pool(name="psA1", bufs=1, space="PSUM") as psA1,
            tc.tile_pool(name="psA2", bufs=1, space="PSUM") as psA2,
            tc.tile_pool(name="psA3", bufs=1, space="PSUM") as psA3,
            tc.tile_pool(name="psBx", bufs=1, space="PSUM") as psBx,
            tc.tile_pool(name="psBy", bufs=1, space="PSUM") as psBy,
            tc.tile_pool(name="psP", bufs=1, space="PSUM") as psP,
            nc.allow_low_precision(reason="fp16 pipeline, tolerance 2e-2"),
        ):
            # ---- constants first on the sync HWDGE ring so phase A can
            # start as soon as the first rgb block lands
            ba_t = cpool.tile([128, 3 * 1088], F16, tag="ba")
            nc.sync.dma_start(ba_t[:], ba[:])
            bb_t = cpool.tile([128, 2 * N_WC * 120], F16, tag="bb")
            nc.sync.dma_start(bb_t[:], bb_[:])
            p4_t = cpool.tile([128, 30], F16, tag="p4")
            nc.sync.dma_start(p4_t[:], p4[:])

            # ---- input loads: SWDGE casts f32->fp16 in flight.
            # image 0 is loaded in W-halves (all blocks' first halves land
            # first, so phase A chunks 0-3 start ~5us earlier); image 1 as
            # whole blocks.  Halves overlap 16 cols for the +-4 w margins.
            def load_block(rgb, b, r0, r1, wl, wr_):
                k = r1 - r0
                nc.gpsimd.dma_start(
                    rgb[0:k, :].rearrange("p (c w) -> p c w", c=3)[:, :, wl:wr_],
                    x[b, :, r0:r1, wl:wr_].rearrange("c p w -> p c w"),
                )

            def emit_loads(b, splits):
                rgbs = [rgbp.tile([128, 3 * W], F16, tag="rgb", name="rgb")
                        for _ in GB]
                for wl, wr_ in splits:
                    for i, (r0, r1) in enumerate(GB):
                        load_block(rgbs[i], b, r0, r1, wl, wr_)
                return rgbs

            # image 0 in W-quarters (phase A starts after 1/4 of the image),
            # image 1 in W-halves (its first chunks start right as image 0's
            # compute drains, instead of waiting for the whole image)
            rgbs_all = {
                0: emit_loads(0, [(0, 128), (116, 244), (236, 484),
                                  (476, 724), (716, W)]),
                1: emit_loads(1, [(0, 488), (472, W)]),
            }

            bias_m1 = cpool.tile([128, 1], F32, tag="bm1")
            nc.gpsimd.memset(bias_m1[:], -1.0)

            stores = []

            for b in range(B_LOC):
                rgb_t = rgbs_all[b]
                pooled = psP.tile([128, 2 * WP], F32, tag="pooled")

                # per-block col offset inside the packed 1088 layout
                AOFF = [0, 228, 456, 684, 912]

                def stage_a(j):
                    """phase A: 5 slots x 3 channel-accumulated matmuls.
                    Emission order 0,1,4,2,3 matches the copy order, so each
                    next-chunk matmul's psum slot is freed by the time the
                    PE queue head reaches it."""
                    w0, w1 = _wj(j)
                    mj = w1 - w0
                    a01 = psA1.tile([128, 456], F32, tag="a01")
                    a23 = psA2.tile([128, 456], F32, tag="a23")
                    a4 = psA3.tile([128, 176], F32, tag="a4")
                    for i in (0, 1, 4, 2, 3):
                        (r0, r1), (s0, s1) = GB[i], SW[i]
                        k = r1 - r0
                        wid = 2 * (s1 - s0)
                        off = AOFF[i]
                        if i < 2:
                            dst = a01[0:mj, off : off + wid]
                        elif i < 4:
                            dst = a23[0:mj, off - 456 : off - 456 + wid]
                        else:
                            dst = a4[0:mj, 0:wid]
                        for c in range(3):
                            nc.tensor.matmul(
                                dst,
                                rgb_t[i][0:k, c * W + w0 : c * W + w1],
                                ba_t[0:k, c * 1088 + off : c * 1088 + off + wid],
                                start=(c == 0),
                                stop=(c == 2),
                            )
                    return a01, a23, a4

                def stage_copy(j, a01, a23, a4):
                    """psum -> sbuf xy fp16 on DVE; single-buffered slots
                    (a4, a01) first so the next chunk's matmuls unblock."""
                    w0, w1 = _wj(j)
                    mj = w1 - w0
                    xy = xyp.tile([128, 1088], F16, tag="xy")
                    nc.vector.tensor_copy(xy[0:mj, 912:1088], a4[0:mj, :])
                    nc.vector.tensor_copy(xy[0:mj, 0:456], a01[0:mj, :])
                    nc.vector.tensor_copy(xy[0:mj, 456:912], a23[0:mj, :])
                    return xy

                def stage_b(j, xy):
                    """phase B per tap into its own psum tile; the Square of
                    gx is emitted between the gx and gy matmuls so it runs on
                    ACT while the PE streams gy -- both tap tiles are free by
                    the time the next chunk's B matmuls reach the PE head."""
                    w0, w1 = _wj(j)
                    mj = w1 - w0
                    xyv = xy[0:mj, :].rearrange("p (s two) -> p two s", two=2)
                    sqs = []
                    for t, ps in ((0, psBx), (1, psBy)):
                        g = ps.tile([128, 768], F32, tag=f"g{t}", name=f"g{t}")
                        bT = bb_t[
                            0:mj, (t * N_WC + j) * 120 : (t * N_WC + j + 1) * 120
                        ]
                        nc.tensor.matmul(
                            g[0:120, 224:512], bT, xyv[:, t, 0:288],
                            start=True, stop=True,
                        )
                        nc.tensor.matmul(
                            g[0:120, 512:768], bT, xyv[:, t, 288:H],
                            start=True, stop=True,
                        )
                        sq = sqp.tile([128, H], F16, tag=f"sq{t}", name=f"sq{t}")
                        nc.scalar.activation(
                            sq[0:120, :], g[0:120, 224:768], AF.Square
                        )
                        sqs.append(sq)
                    m2 = sqp.tile([128, H], F16, tag="m2")
                    # image 0: DVE (POOL is still emitting SWDGE loads);
                    # image 1: POOL, so the DVE FIFO reaches the psum-freeing
                    # casts sooner and the PE queue head never waits on them
                    eng = nc.vector if b == 0 else nc.gpsimd
                    eng.tensor_add(
                        m2[0:120, :], sqs[0][0:120, :], sqs[1][0:120, :]
                    )
                    return m2

                def stage_mag2(j, m2):
                    mg = sqp.tile([128, H], F16, tag="mg")
                    nc.scalar.activation(mg[0:120, :], m2[0:120, :], AF.Sqrt)
                    sp = spp.tile([128, HP], F16, tag="sp")
                    nc.vector.tensor_reduce(
                        sp[0:120, :],
                        mg[0:120, :].rearrange("p (g f) -> p g f", f=4),
                        axis=mybir.AxisListType.X,
                        op=ALU.add,
                    )
                    return sp

                def stage_pool(j, sp):
                    nc.tensor.matmul(
                        pooled[0:96, 30 * j : 30 * j + 30],
                        sp[0:120, 0:96],
                        p4_t[0:120, :],
                        start=True,
                        stop=True,
                    )
                    nc.tensor.matmul(
                        pooled[0:40, WP + 30 * j : WP + 30 * j + 30],
                        sp[0:120, 96:HP],
                        p4_t[0:120, :],
                        start=True,
                        stop=True,
                    )

                # software-pipelined emission: PE queue order pool(j-1),
                # A(j+1), B(j); the whole mag chain (Square, add, sqrt,
                # reduce) runs within iteration j so the drain tail is short
                # and the single psB buffer frees before B(j+1) hits the PE
                aout = {0: stage_a(0)}
                m2s, sps = {}, {}
                for j in range(N_WC + 3):
                    if 0 <= j < N_WC:
                        xy = stage_copy(j, *aout.pop(j))
                    if j - 3 in sps:
                        stage_pool(j - 3, sps.pop(j - 3))
                    if j + 1 < N_WC:
                        aout[j + 1] = stage_a(j + 1)
                    if 0 <= j < N_WC:
                        m2s[j] = stage_b(j, xy)
                    if j - 1 in m2s:
                        sps[j - 1] = stage_mag2(j - 1, m2s.pop(j - 1))

                # ---- sigmoid(5x-1)^2 on pooled, store deferred to the end
                sg = outp.tile([128, 2 * WP], F32, tag="sg")
                nc.scalar.activation(
                    sg[0:96, :], pooled[0:96, :], AF.Sigmoid,
                    bias=bias_m1[0:96, :], scale=5.0,
                )
                ot = outp.tile([128, 2 * WP], F32, tag="ot")
                nc.gpsimd.tensor_mul(ot[0:96, :], sg[0:96, :], sg[0:96, :])
                stores.append((b, ot))

            for b, ot in stores:
                nc.sync.dma_start(y[b, 0, 0:96, :], ot[0:96, 0:WP])
                nc.sync.dma_start(y[b, 0, 96:HP, :], ot[0:40, WP : 2 * WP])

    split_multi_waits(nc)
    return nc


_NC = None
_CONSTS = None
TRACE = False
LAST_EXEC_NS = None


def kernel(**inputs):
    global _NC, _CONSTS, LAST_EXEC_NS
    left_rgb = np.ascontiguousarray(np.asarray(inputs["left_rgb"], dtype=np.float32))
    assert left_rgb.shape == (B_FULL, C, H, W)
    if _NC is None:
        _NC = build_module()
        _CONSTS = build_constants()
    band_a, band_b, p4 = _CONSTS
    in_maps = [
        {
            "x": np.ascontiguousarray(left_rgb[i * B_LOC : (i + 1) * B_LOC]),
            "bA": band_a,
            "bB": band_b,
            "p4": p4,
        }
        for i in range(N_CORES)
    ]
    res = run_bass_kernel_spmd(
        _NC, in_maps, core_ids=list(range(N_CORES)), trace=TRACE
    )
    LAST_EXEC_NS = res.exec_time_ns
    out = np.empty((B_FULL, 1, HP, WP), dtype=np.float32)
    for i in range(N_CORES):
        out[i * B_LOC : (i + 1) * B_LOC] = res.results[i]["y"]
    return out


# revision 48
# speedup vs baseline: 1.2354x; 1.0815x over previous
"""EdgeGuidance Trainium2 kernel.

Pipeline per image [3,544,960] -> [1,136,240]:
  gray = w.RGB  ->  smooth = gauss5x5(reflect)  ->  gx,gy = sobel(zero-pad)
  mag = sqrt(gx^2+gy^2)  ->  4x4 avgpool  ->  sigmoid(5(x-0.2))^2

All linear steps fold into two banded-matrix passes on the PE in fp16
(1 cycle/row; rel err ~2e-3 vs 2e-2 budget):
  gx = A_x @ gray @ Bx^T,   gy = A_y @ gray @ By^T
There is NO explicit gray pass: inputs are cast f32->fp16 during the
SWDGE DMA itself, and the channel weights are folded into three
pre-scaled copies of the phase-A band; phase A accumulates the three
channel matmuls in PSUM.  Phase A uses the rgb slab as the matmul
stationary so its output lands transposed ([w, s]); each of 5 row-blocks
owns a disjoint s-window (rows overlap by 6 so no cross-block PSUM
accumulation is needed).  Phase B contracts over w with the B^T band
stationary.

The cast-during-DMA halves the SBUF-side fabric traffic, so the input
stream sustains ~345 GB/s read-side (~38 us for 13.3 MB); the PE
(~47 us of LDWEIGHTS+MATMUL) is the binding engine, so the schedule is
built around keeping its queue head unblocked: single-buffered psum
slots are copied out in the same order the next chunk's matmuls are
emitted, and each tap's Square runs on ACT while the PE streams the
other tap.  Elementwise tail: DVE does all psum->sbuf fp16 copies, the
fp16 m2 add (2x mode) and the 4x reduce; ACT does Square/Sqrt/sigmoid;
POOL does SWDGE issue + the final squares.  Image 0 loads in W-eighth/
quarter steps (compute starts after ~1 MB), image 1 in halves (its
chunks start right as image 0 drains).

Data parallel over batch: 8 cores x 2 images.
"""

import numpy as np

import concourse.bass as bass
import concourse.tile as tile
from concourse import mybir
from concourse.bass_utils import run_bass_kernel_spmd

F32 = mybir.dt.float32
F16 = mybir.dt.float16
AF = mybir.ActivationFunctionType
ALU = mybir.AluOpType

B_FULL, C, H, W = 16, 3, 544, 960
N_CORES = 8
B_LOC = B_FULL // N_CORES  # images per core
HP, WP = H // 4, W // 4  # 136, 240

BLUR_K, SIGMA = 5, 1.5
W_R, W_G, W_B = 0.2989, 0.587, 0.114

# 5 row-blocks (k multiple of 8 for DMA engine spray), each owning a
# disjoint s-window; rows [s-3, s+4) of every owned s lie inside the block.
GB = [(0, 120), (111, 231), (225, 345), (339, 459), (448, 544)]
SW = [(0, 114), (114, 228), (228, 342), (342, 456), (456, 544)]
# phase-A psum packing: slots 0,1 in a01 [456], slots 2,3 in a23 [456],
# slot 4 in a4 [176], each 1 bank x bufs=1; phase-B taps gx/gy get a
# 2-bank tile each -- 1+1+1 + 2+2 + psP 1 = 8 banks exactly.
N_WC = 8  # w-chunks of 120 outputs each


def _wj(j):
    return max(0, 120 * j - 4), min(W, 120 * j + 124)


# ---------------------------------------------------------------- numpy bands
def _blur1d():
    x = np.arange(BLUR_K, dtype=np.float64) - (BLUR_K - 1) / 2.0
    g = np.exp(-(x**2) / (2.0 * SIGMA**2))
    return g / g.sum()


def _band_reflect(n, taps):
    r = len(taps) // 2
    m = np.zeros((n, n), dtype=np.float64)
    for s in range(n):
        for d in range(-r, r + 1):
            i = s + d
            if i < 0:
                i = -i
            elif i >= n:
                i = 2 * n - 2 - i
            m[s, i] += taps[d + r]
    return m


def _band_zero(n, taps):
    r = len(taps) // 2
    m = np.zeros((n, n), dtype=np.float64)
    for s in range(n):
        for d in range(-r, r + 1):
            i = s + d
            if 0 <= i < n:
                m[s, i] += taps[d + r]
    return m


def build_constants():
    f16 = np.float16
    g1 = _blur1d()
    vb_h = _band_reflect(H, g1)  # vertical blur on H
    hb_w = _band_reflect(W, g1)  # horizontal blur on W
    ax = _band_zero(H, [1.0, 2.0, 1.0]) @ vb_h
    ay = _band_zero(H, [-1.0, 0.0, 1.0]) @ vb_h
    bx = _band_zero(W, [-1.0, 0.0, 1.0]) @ hb_w
    by = _band_zero(W, [1.0, 2.0, 1.0]) @ hb_w

    # phase A: per channel c (scaled by its gray weight), 5 slots packed
    # back-to-back, interleaved (s, t): col c*1088 + off_i + 2u + t
    band_a = np.zeros((128, 3 * 1088), dtype=np.float64)
    for c, wc in enumerate((W_R, W_G, W_B)):
        off = 0
        for (r0, r1), (s0, s1) in zip(GB, SW):
            k, w_ = r1 - r0, s1 - s0
            blk = np.stack(
                [wc * ax[s0:s1, r0:r1], wc * ay[s0:s1, r0:r1]], axis=-1
            )  # [w,k,2]
            band_a[0:k, c * 1088 + off : c * 1088 + off + 2 * w_] = blk.transpose(
                1, 0, 2
            ).reshape(k, 2 * w_)
            off += 2 * w_

    # phase B: per (t, j) block [mj, 120] at cols (t*8+j)*120
    band_b = np.zeros((128, 2 * N_WC * 120), dtype=np.float64)
    for t, m in enumerate((bx, by)):
        for j in range(N_WC):
            w0, w1 = _wj(j)
            blk = m[120 * j : 120 * j + 120, w0:w1].T  # [mj, 120]
            band_b[0 : w1 - w0, (t * N_WC + j) * 120 : (t * N_WC + j + 1) * 120] = blk

    p4 = np.zeros((128, 30), dtype=np.float64)
    for wp in range(120):
        p4[wp, wp // 4] = 1.0 / 16.0
    return (
        band_a.astype(f16),
        band_b.astype(f16),
        p4.astype(f16),
    )


# ------------------------------------------------------------------ bass build
def split_multi_waits(nc):
    """walrus in this container only accepts 1 sync-wait per instruction;
    hoist extra waits onto preceding same-engine NoOps."""
    for fn in nc.m.functions:
        for bb in fn.blocks:
            new_list, changed = [], False
            for ins in bb.instructions:
                si = ins.sync_info
                waits = list(si.on_wait) if si is not None else []
                if len(waits) > 1:
                    changed = True
                    for i, wt in enumerate(waits[:-1]):
                        new_list.append(
                            mybir.InstNoOp(
                                name=f"{ins.name}_ws{i}",
                                engine=ins.engine,
                                bass_nofuse=True,
                                sync_info=mybir.SyncInfo(on_wait=[wt], on_update=[]),
                            )
                        )
                    si.on_wait = [waits[-1]]
                    ins.sync_info = si
                new_list.append(ins)
            if changed:
                bb.instructions = new_list


def build_module():
    nc = bass.Bass("TRN2", target_bir_lowering=False, debug=False)
    x = nc.dram_tensor("x", [B_LOC, C, H, W], F32, kind="ExternalInput").ap()
    ba = nc.dram_tensor("bA", [128, 3 * 1088], F16, kind="ExternalInput").ap()
    bb_ = nc.dram_tensor("bB", [128, 2 * N_WC * 120], F16, kind="ExternalInput").ap()
    p4 = nc.dram_tensor("p4", [128, 30], F16, kind="ExternalInput").ap()
    y = nc.dram_tensor("y", [B_LOC, 1, HP, WP], F32, kind="ExternalOutput").ap()

    with tile.TileContext(nc) as tc:
        with (
            tc.tile_pool(name="const", bufs=1) as cpool,
            tc.tile_pool(name="rgb", bufs=10) as rgbp,
            tc.tile_pool(name="xy", bufs=6) as xyp,
            tc.tile_pool(name="sq", bufs=3) as sqp,
            tc.tile_pool(name="sp", bufs=3) as spp,
            tc.tile_pool(name="outp", bufs=2) as outp,
            tc.tile_pool(name="psA1", bufs=1, space="PSUM") as psA1,
            tc.tile_pool(name="psA2", bufs=1, space="PSUM") as psA2,
            tc.tile_pool(name="psA3", bufs=1, space="PSUM") as psA3,
            tc.tile_pool(name="psBx", bufs=1, space="PSUM") as psBx,
            tc.tile_pool(name="psBy", bufs=1, space="PSUM") as psBy,
            tc.tile_pool(name="psP", bufs=1, space="PSUM") as psP,
            nc.allow_low_precision(reason="fp16 pipeline, tolerance 2e-2"),
        ):
            # ---- constants first on the sync HWDGE ring so phase A can
            # start as soon as the first rgb block lands
            ba_t = cpool.tile([128, 3 * 1088], F16, tag="ba")
            nc.sync.dma_start(ba_t[:], ba[:])
            bb_t = cpool.tile([128, 2 * N_WC * 120], F16, tag="bb")
            nc.sync.dma_start(bb_t[:], bb_[:])
            p4_t = cpool.tile([128, 30], F16, tag="p4")
            nc.sync.dma_start(p4_t[:], p4[:])

            # ---- input loads: SWDGE casts f32->fp16 in flight.
            # image 0 is loaded in W-halves (all blocks' first halves land
            # first, so phase A chunks 0-3 start ~5us earlier); image 1 as
            # whole blocks.  Halves overlap 16 cols for the +-4 w margins.
            def load_block(rgb, b, r0, r1, wl, wr_):
                k = r1 - r0
                nc.gpsimd.dma_start(
                    rgb[0:k, :].rearrange("p (c w) -> p c w", c=3)[:, :, wl:wr_],
                    x[b, :, r0:r1, wl:wr_].rearrange("c p w -> p c w"),
                )

            def emit_loads(b, splits):
                rgbs = [rgbp.tile([128, 3 * W], F16, tag="rgb", name="rgb")
                        for _ in GB]
                for wl, wr_ in splits:
                    for i, (r0, r1) in enumerate(GB):
                        load_block(rgbs[i], b, r0, r1, wl, wr_)
                return rgbs

            # image 0 in W-quarters (phase A starts after 1/4 of the image),
            # image 1 in W-halves (its first chunks start right as image 0's
            # compute drains, instead of waiting for the whole image)
            rgbs_all = {
                0: emit_loads(0, [(0, 244), (236, 484), (476, 724), (716, W)]),
                1: emit_loads(1, [(0, 488), (472, W)]),
            }

            bias_m1 = cpool.tile([128, 1], F32, tag="bm1")
            nc.gpsimd.memset(bias_m1[:], -1.0)

            stores = []

            for b in range(B_LOC):
                rgb_t = rgbs_all[b]
                pooled = psP.tile([128, 2 * WP], F32, tag="pooled")

                # per-block col offset inside the packed 1088 layout
                AOFF = [0, 228, 456, 684, 912]

                def stage_a(j):
                    """phase A: 5 slots x 3 channel-accumulated matmuls.
                    Emission order 0,1,4,2,3 matches the copy order, so each
                    next-chunk matmul's psum slot is freed by the time the
                    PE queue head reaches it."""
                    w0, w1 = _wj(j)
                    mj = w1 - w0
                    a01 = psA1.tile([128, 456], F32, tag="a01")
                    a23 = psA2.tile([128, 456], F32, tag="a23")
                    a4 = psA3.tile([128, 176], F32, tag="a4")
                    for i in (0, 1, 4, 2, 3):
                        (r0, r1), (s0, s1) = GB[i], SW[i]
                        k = r1 - r0
                        wid = 2 * (s1 - s0)
                        off = AOFF[i]
                        if i < 2:
                            dst = a01[0:mj, off : off + wid]
                        elif i < 4:
                            dst = a23[0:mj, off - 456 : off - 456 + wid]
                        else:
                            dst = a4[0:mj, 0:wid]
                        for c in range(3):
                            nc.tensor.matmul(
                                dst,
                                rgb_t[i][0:k, c * W + w0 : c * W + w1],
                                ba_t[0:k, c * 1088 + off : c * 1088 + off + wid],
                                start=(c == 0),
                                stop=(c == 2),
                            )
                    return a01, a23, a4

                def stage_copy(j, a01, a23, a4):
                    """psum -> sbuf xy fp16 on DVE; single-buffered slots
                    (a4, a01) first so the next chunk's matmuls unblock."""
                    w0, w1 = _wj(j)
                    mj = w1 - w0
                    xy = xyp.tile([128, 1088], F16, tag="xy")
                    nc.vector.tensor_copy(xy[0:mj, 912:1088], a4[0:mj, :])
                    nc.vector.tensor_copy(xy[0:mj, 0:456], a01[0:mj, :])
                    nc.vector.tensor_copy(xy[0:mj, 456:912], a23[0:mj, :])
                    return xy

                def stage_b(j, xy):
                    """phase B per tap into its own psum tile; the Square of
                    gx is emitted between the gx and gy matmuls so it runs on
                    ACT while the PE streams gy -- both tap tiles are free by
                    the time the next chunk's B matmuls reach the PE head."""
                    w0, w1 = _wj(j)
                    mj = w1 - w0
                    xyv = xy[0:mj, :].rearrange("p (s two) -> p two s", two=2)
                    sqs = []
                    for t, ps in ((0, psBx), (1, psBy)):
                        g = ps.tile([128, 768], F32, tag=f"g{t}", name=f"g{t}")
                        bT = bb_t[
                            0:mj, (t * N_WC + j) * 120 : (t * N_WC + j + 1) * 120
                        ]
                        nc.tensor.matmul(
                            g[0:120, 224:512], bT, xyv[:, t, 0:288],
                            start=True, stop=True,
                        )
                        nc.tensor.matmul(
                            g[0:120, 512:768], bT, xyv[:, t, 288:H],
                            start=True, stop=True,
                        )
                        sq = sqp.tile([128, H], F16, tag=f"sq{t}", name=f"sq{t}")
                        nc.scalar.activation(
                            sq[0:120, :], g[0:120, 224:768], AF.Square
                        )
                        sqs.append(sq)
                    m2 = sqp.tile([128, H], F16, tag="m2")
                    # image 0: DVE (POOL is still emitting SWDGE loads);
                    # image 1: POOL, so the DVE FIFO reaches the psum-freeing
                    # casts sooner and the PE queue head never waits on them
                    eng = nc.vector if b == 0 else nc.gpsimd
                    eng.tensor_add(
                        m2[0:120, :], sqs[0][0:120, :], sqs[1][0:120, :]
                    )
                    return m2

                def stage_mag2(j, m2):
                    mg = sqp.tile([128, H], F16, tag="mg")
                    nc.scalar.activation(mg[0:120, :], m2[0:120, :], AF.Sqrt)
                    sp = spp.tile([128, HP], F16, tag="sp")
                    nc.vector.tensor_reduce(
                        sp[0:120, :],
                        mg[0:120, :].rearrange("p (g f) -> p g f", f=4),
                        axis=mybir.AxisListType.X,
                        op=ALU.add,
                    )
                    return sp

                def stage_pool(j, sp):
                    nc.tensor.matmul(
                        pooled[0:96, 30 * j : 30 * j + 30],
                        sp[0:120, 0:96],
                        p4_t[0:120, :],
                        start=True,
                        stop=True,
                    )
                    nc.tensor.matmul(
                        pooled[0:40, WP + 30 * j : WP + 30 * j + 30],
                        sp[0:120, 96:HP],
                        p4_t[0:120, :],
                        start=True,
                        stop=True,
                    )

                # software-pipelined emission: PE queue order pool(j-1),
                # A(j+1), B(j); the whole mag chain (Square, add, sqrt,
                # reduce) runs within iteration j so the drain tail is short
                # and the single psB buffer frees before B(j+1) hits the PE
                aout = {0: stage_a(0)}
                m2s, sps = {}, {}
                for j in range(N_WC + 3):
                    if 0 <= j < N_WC:
                        xy = stage_copy(j, *aout.pop(j))
                    if j - 3 in sps:
                        stage_pool(j - 3, sps.pop(j - 3))
                    if j + 1 < N_WC:
                        aout[j + 1] = stage_a(j + 1)
                    if 0 <= j < N_WC:
                        m2s[j] = stage_b(j, xy)
                    if j - 1 in m2s:
                        sps[j - 1] = stage_mag2(j - 1, m2s.pop(j - 1))

                # ---- sigmoid(5x-1)^2 on pooled, store deferred to the end
                sg = outp.tile([128, 2 * WP], F32, tag="sg")
                nc.scalar.activation(
                    sg[0:96, :], pooled[0:96, :], AF.Sigmoid,
                    bias=bias_m1[0:96, :], scale=5.0,
                )
                ot = outp.tile([128, 2 * WP], F32, tag="ot")
                nc.gpsimd.tensor_mul(ot[0:96, :], sg[0:96, :], sg[0:96, :])
                stores.append((b, ot))

            for b, ot in stores:
                nc.sync.dma_start(y[b, 0, 0:96, :], ot[0:96, 0:WP])
                nc.sync.dma_start(y[b, 0, 96:HP, :], ot[0:40, WP : 2 * WP])

    split_multi_waits(nc)
    return nc


_NC = None
_CONSTS = None
TRACE = False
LAST_EXEC_NS = None


def kernel(**inputs):
    global _NC, _CONSTS, LAST_EXEC_NS
    left_rgb = np.ascontiguousarray(np.asarray(inputs["left_rgb"], dtype=np.float32))
    assert left_rgb.shape == (B_FULL, C, H, W)
    if _NC is None:
        _NC = build_module()
        _CONSTS = build_constants()
    band_a, band_b, p4 = _CONSTS
    in_maps = [
        {
            "x": np.ascontiguousarray(left_rgb[i * B_LOC : (i + 1) * B_LOC]),
            "bA": band_a,
            "bB": band_b,
            "p4": p4,
        }
        for i in range(N_CORES)
    ]
    res = run_bass_kernel_spmd(
        _NC, in_maps, core_ids=list(range(N_CORES)), trace=TRACE
    )
    LAST_EXEC_NS = res.exec_time_ns
    out = np.empty((B_FULL, 1, HP, WP), dtype=np.float32)
    for i in range(N_CORES):
        out[i * B_LOC : (i + 1) * B_LOC] = res.results[i]["y"]
    return out


# revision 49
# speedup vs baseline: 1.2671x; 1.0256x over previous
"""EdgeGuidance Trainium2 kernel.

Pipeline per image [3,544,960] -> [1,136,240]:
  gray = w.RGB  ->  smooth = gauss5x5(reflect)  ->  gx,gy = sobel(zero-pad)
  mag = sqrt(gx^2+gy^2)  ->  4x4 avgpool  ->  sigmoid(5(x-0.2))^2

All linear steps fold into two banded-matrix passes on the PE in fp16
(1 cycle/row; rel err ~2e-3 vs 2e-2 budget):
  gx = A_x @ gray @ Bx^T,   gy = A_y @ gray @ By^T
There is NO explicit gray pass: inputs are cast f32->fp16 during the
SWDGE DMA itself, and the channel weights are folded into three
pre-scaled copies of the phase-A band; phase A accumulates the three
channel matmuls in PSUM.  Phase A uses the rgb slab as the matmul
stationary so its output lands transposed ([w, s]); each of 5 row-blocks
owns a disjoint s-window (rows overlap by 6 so no cross-block PSUM
accumulation is needed).  Phase B contracts over w with the B^T band
stationary.

The cast-during-DMA halves the SBUF-side fabric traffic, so the input
stream sustains ~345 GB/s read-side (~38 us for 13.3 MB); the PE
(~47 us of LDWEIGHTS+MATMUL) is the binding engine, so the schedule is
built around keeping its queue head unblocked: single-buffered psum
slots are copied out in the same order the next chunk's matmuls are
emitted, and each tap's Square runs on ACT while the PE streams the
other tap.  Elementwise tail: DVE does all psum->sbuf fp16 copies, the
fp16 m2 add (2x mode) and the 4x reduce; ACT does Square/Sqrt/sigmoid;
POOL does SWDGE issue + the final squares.  Image 0 loads in W-eighth/
quarter steps (compute starts after ~1 MB), image 1 in halves (its
chunks start right as image 0 drains).

Data parallel over batch: 8 cores x 2 images.
"""

import numpy as np

import concourse.bass as bass
import concourse.tile as tile
from concourse import mybir
from concourse.bass_utils import run_bass_kernel_spmd

F32 = mybir.dt.float32
F16 = mybir.dt.float16
AF = mybir.ActivationFunctionType
ALU = mybir.AluOpType

B_FULL, C, H, W = 16, 3, 544, 960
N_CORES = 8
B_LOC = B_FULL // N_CORES  # images per core
HP, WP = H // 4, W // 4  # 136, 240

BLUR_K, SIGMA = 5, 1.5
W_R, W_G, W_B = 0.2989, 0.587, 0.114

# 5 row-blocks (k multiple of 8 for DMA engine spray), each owning a
# disjoint s-window; rows [s-3, s+4) of every owned s lie inside the block.
GB = [(0, 120), (111, 231), (225, 345), (339, 459), (448, 544)]
SW = [(0, 114), (114, 228), (228, 342), (342, 456), (456, 544)]
# phase-A psum packing: slots 0,1 in a01 [456], slots 2,3 in a23 [456],
# slot 4 in a4 [176], each 1 bank x bufs=1; phase-B taps gx/gy get a
# 2-bank tile each -- 1+1+1 + 2+2 + psP 1 = 8 banks exactly.
N_WC = 8  # w-chunks of 120 outputs each


def _wj(j):
    return max(0, 120 * j - 4), min(W, 120 * j + 124)


# ---------------------------------------------------------------- numpy bands
def _blur1d():
    x = np.arange(BLUR_K, dtype=np.float64) - (BLUR_K - 1) / 2.0
    g = np.exp(-(x**2) / (2.0 * SIGMA**2))
    return g / g.sum()


def _band_reflect(n, taps):
    r = len(taps) // 2
    m = np.zeros((n, n), dtype=np.float64)
    for s in range(n):
        for d in range(-r, r + 1):
            i = s + d
            if i < 0:
                i = -i
            elif i >= n:
                i = 2 * n - 2 - i
            m[s, i] += taps[d + r]
    return m


def _band_zero(n, taps):
    r = len(taps) // 2
    m = np.zeros((n, n), dtype=np.float64)
    for s in range(n):
        for d in range(-r, r + 1):
            i = s + d
            if 0 <= i < n:
                m[s, i] += taps[d + r]
    return m


def build_constants():
    f16 = np.float16
    g1 = _blur1d()
    vb_h = _band_reflect(H, g1)  # vertical blur on H
    hb_w = _band_reflect(W, g1)  # horizontal blur on W
    ax = _band_zero(H, [1.0, 2.0, 1.0]) @ vb_h
    ay = _band_zero(H, [-1.0, 0.0, 1.0]) @ vb_h
    bx = _band_zero(W, [-1.0, 0.0, 1.0]) @ hb_w
    by = _band_zero(W, [1.0, 2.0, 1.0]) @ hb_w

    # phase A: per channel c (scaled by its gray weight), 5 slots packed
    # back-to-back, interleaved (s, t): col c*1088 + off_i + 2u + t
    band_a = np.zeros((128, 3 * 1088), dtype=np.float64)
    for c, wc in enumerate((W_R, W_G, W_B)):
        off = 0
        for (r0, r1), (s0, s1) in zip(GB, SW):
            k, w_ = r1 - r0, s1 - s0
            blk = np.stack(
                [wc * ax[s0:s1, r0:r1], wc * ay[s0:s1, r0:r1]], axis=-1
            )  # [w,k,2]
            band_a[0:k, c * 1088 + off : c * 1088 + off + 2 * w_] = blk.transpose(
                1, 0, 2
            ).reshape(k, 2 * w_)
            off += 2 * w_

    # phase B: per (t, j) block [mj, 120] at cols (t*8+j)*120
    band_b = np.zeros((128, 2 * N_WC * 120), dtype=np.float64)
    for t, m in enumerate((bx, by)):
        for j in range(N_WC):
            w0, w1 = _wj(j)
            blk = m[120 * j : 120 * j + 120, w0:w1].T  # [mj, 120]
            band_b[0 : w1 - w0, (t * N_WC + j) * 120 : (t * N_WC + j + 1) * 120] = blk

    p4 = np.zeros((128, 30), dtype=np.float64)
    for wp in range(120):
        p4[wp, wp // 4] = 1.0 / 16.0
    return (
        band_a.astype(f16),
        band_b.astype(f16),
        p4.astype(f16),
    )


# ------------------------------------------------------------------ bass build
def split_multi_waits(nc):
    """walrus in this container only accepts 1 sync-wait per instruction;
    hoist extra waits onto preceding same-engine NoOps."""
    for fn in nc.m.functions:
        for bb in fn.blocks:
            new_list, changed = [], False
            for ins in bb.instructions:
                si = ins.sync_info
                waits = list(si.on_wait) if si is not None else []
                if len(waits) > 1:
                    changed = True
                    for i, wt in enumerate(waits[:-1]):
                        new_list.append(
                            mybir.InstNoOp(
                                name=f"{ins.name}_ws{i}",
                                engine=ins.engine,
                                bass_nofuse=True,
                                sync_info=mybir.SyncInfo(on_wait=[wt], on_update=[]),
                            )
                        )
                    si.on_wait = [waits[-1]]
                    ins.sync_info = si
                new_list.append(ins)
            if changed:
                bb.instructions = new_list


def build_module():
    nc = bass.Bass("TRN2", target_bir_lowering=False, debug=False)
    x = nc.dram_tensor("x", [B_LOC, C, H, W], F32, kind="ExternalInput").ap()
    ba = nc.dram_tensor("bA", [128, 3 * 1088], F16, kind="ExternalInput").ap()
    bb_ = nc.dram_tensor("bB", [128, 2 * N_WC * 120], F16, kind="ExternalInput").ap()
    p4 = nc.dram_tensor("p4", [128, 30], F16, kind="ExternalInput").ap()
    y = nc.dram_tensor("y", [B_LOC, 1, HP, WP], F32, kind="ExternalOutput").ap()

    with tile.TileContext(nc) as tc:
        with (
            tc.tile_pool(name="const", bufs=1) as cpool,
            tc.tile_pool(name="rgb", bufs=10) as rgbp,
            tc.tile_pool(name="xy", bufs=6) as xyp,
            tc.tile_pool(name="sq", bufs=3) as sqp,
            tc.tile_pool(name="sp", bufs=3) as spp,
            tc.tile_pool(name="outp", bufs=2) as outp,
            tc.tile_pool(name="psA1", bufs=1, space="PSUM") as psA1,
            tc.tile_pool(name="psA2", bufs=1, space="PSUM") as psA2,
            tc.tile_pool(name="psA3", bufs=1, space="PSUM") as psA3,
            tc.tile_pool(name="psBx", bufs=1, space="PSUM") as psBx,
            tc.tile_pool(name="psBy", bufs=1, space="PSUM") as psBy,
            tc.tile_pool(name="psP", bufs=1, space="PSUM") as psP,
            nc.allow_low_precision(reason="fp16 pipeline, tolerance 2e-2"),
        ):
            # ---- constants first on the sync HWDGE ring so phase A can
            # start as soon as the first rgb block lands
            # ba (the phase-A gate) alone on the sync ring so it lands
            # first; bb/p4 ride the scalar ring in parallel (ACT is idle
            # until the first Square at ~17us)
            ba_t = cpool.tile([128, 3 * 1088], F16, tag="ba")
            nc.sync.dma_start(ba_t[:], ba[:])
            bb_t = cpool.tile([128, 2 * N_WC * 120], F16, tag="bb")
            nc.scalar.dma_start(bb_t[:], bb_[:])
            p4_t = cpool.tile([128, 30], F16, tag="p4")
            nc.scalar.dma_start(p4_t[:], p4[:])

            # ---- input loads: SWDGE casts f32->fp16 in flight.
            # image 0 is loaded in W-halves (all blocks' first halves land
            # first, so phase A chunks 0-3 start ~5us earlier); image 1 as
            # whole blocks.  Halves overlap 16 cols for the +-4 w margins.
            def load_block(rgb, b, r0, r1, wl, wr_):
                k = r1 - r0
                nc.gpsimd.dma_start(
                    rgb[0:k, :].rearrange("p (c w) -> p c w", c=3)[:, :, wl:wr_],
                    x[b, :, r0:r1, wl:wr_].rearrange("c p w -> p c w"),
                )

            def emit_loads(b, splits):
                rgbs = [rgbp.tile([128, 3 * W], F16, tag="rgb", name="rgb")
                        for _ in GB]
                for wl, wr_ in splits:
                    for i, (r0, r1) in enumerate(GB):
                        load_block(rgbs[i], b, r0, r1, wl, wr_)
                return rgbs

            # image 0 in W-quarters (phase A starts after 1/4 of the image),
            # image 1 in W-halves (its first chunks start right as image 0's
            # compute drains, instead of waiting for the whole image)
            rgbs_all = {
                0: emit_loads(0, [(0, 244), (236, 484), (476, 724), (716, W)]),
                1: emit_loads(1, [(0, 488), (472, W)]),
            }

            bias_m1 = cpool.tile([128, 1], F32, tag="bm1")
            nc.gpsimd.memset(bias_m1[:], -1.0)

            stores = []

            for b in range(B_LOC):
                rgb_t = rgbs_all[b]
                pooled = psP.tile([128, 2 * WP], F32, tag="pooled")

                # per-block col offset inside the packed 1088 layout
                AOFF = [0, 228, 456, 684, 912]

                def stage_a(j):
                    """phase A: 5 slots x 3 channel-accumulated matmuls.
                    Emission order 0,1,4,2,3 matches the copy order, so each
                    next-chunk matmul's psum slot is freed by the time the
                    PE queue head reaches it."""
                    w0, w1 = _wj(j)
                    mj = w1 - w0
                    a01 = psA1.tile([128, 456], F32, tag="a01")
                    a23 = psA2.tile([128, 456], F32, tag="a23")
                    a4 = psA3.tile([128, 176], F32, tag="a4")
                    for i in (0, 1, 4, 2, 3):
                        (r0, r1), (s0, s1) = GB[i], SW[i]
                        k = r1 - r0
                        wid = 2 * (s1 - s0)
                        off = AOFF[i]
                        if i < 2:
                            dst = a01[0:mj, off : off + wid]
                        elif i < 4:
                            dst = a23[0:mj, off - 456 : off - 456 + wid]
                        else:
                            dst = a4[0:mj, 0:wid]
                        for c in range(3):
                            nc.tensor.matmul(
                                dst,
                                rgb_t[i][0:k, c * W + w0 : c * W + w1],
                                ba_t[0:k, c * 1088 + off : c * 1088 + off + wid],
                                start=(c == 0),
                                stop=(c == 2),
                            )
                    return a01, a23, a4

                def stage_copy(j, a01, a23, a4):
                    """psum -> sbuf xy fp16 on DVE; single-buffered slots
                    (a4, a01) first so the next chunk's matmuls unblock."""
                    w0, w1 = _wj(j)
                    mj = w1 - w0
                    xy = xyp.tile([128, 1088], F16, tag="xy")
                    nc.vector.tensor_copy(xy[0:mj, 912:1088], a4[0:mj, :])
                    nc.vector.tensor_copy(xy[0:mj, 0:456], a01[0:mj, :])
                    nc.vector.tensor_copy(xy[0:mj, 456:912], a23[0:mj, :])
                    return xy

                def stage_b(j, xy):
                    """phase B per tap into its own psum tile; the Square of
                    gx is emitted between the gx and gy matmuls so it runs on
                    ACT while the PE streams gy -- both tap tiles are free by
                    the time the next chunk's B matmuls reach the PE head."""
                    w0, w1 = _wj(j)
                    mj = w1 - w0
                    xyv = xy[0:mj, :].rearrange("p (s two) -> p two s", two=2)
                    sqs = []
                    for t, ps in ((0, psBx), (1, psBy)):
                        g = ps.tile([128, 768], F32, tag=f"g{t}", name=f"g{t}")
                        bT = bb_t[
                            0:mj, (t * N_WC + j) * 120 : (t * N_WC + j + 1) * 120
                        ]
                        nc.tensor.matmul(
                            g[0:120, 224:512], bT, xyv[:, t, 0:288],
                            start=True, stop=True,
                        )
                        nc.tensor.matmul(
                            g[0:120, 512:768], bT, xyv[:, t, 288:H],
                            start=True, stop=True,
                        )
                        sq = sqp.tile([128, H], F16, tag=f"sq{t}", name=f"sq{t}")
                        nc.scalar.activation(
                            sq[0:120, :], g[0:120, 224:768], AF.Square
                        )
                        sqs.append(sq)
                    m2 = sqp.tile([128, H], F16, tag="m2")
                    # image 0: DVE (POOL is still emitting SWDGE loads);
                    # image 1: POOL, so the DVE FIFO reaches the psum-freeing
                    # casts sooner and the PE queue head never waits on them
                    eng = nc.vector if b == 0 else nc.gpsimd
                    eng.tensor_add(
                        m2[0:120, :], sqs[0][0:120, :], sqs[1][0:120, :]
                    )
                    return m2

                def stage_mag2(j, m2):
                    mg = sqp.tile([128, H], F16, tag="mg")
                    nc.scalar.activation(mg[0:120, :], m2[0:120, :], AF.Sqrt)
                    sp = spp.tile([128, HP], F16, tag="sp")
                    nc.vector.tensor_reduce(
                        sp[0:120, :],
                        mg[0:120, :].rearrange("p (g f) -> p g f", f=4),
                        axis=mybir.AxisListType.X,
                        op=ALU.add,
                    )
                    return sp

                def stage_pool(j, sp):
                    nc.tensor.matmul(
                        pooled[0:96, 30 * j : 30 * j + 30],
                        sp[0:120, 0:96],
                        p4_t[0:120, :],
                        start=True,
                        stop=True,
                    )
                    nc.tensor.matmul(
                        pooled[0:40, WP + 30 * j : WP + 30 * j + 30],
                        sp[0:120, 96:HP],
                        p4_t[0:120, :],
                        start=True,
                        stop=True,
                    )

                # software-pipelined emission: PE queue order pool(j-1),
                # A(j+1), B(j); the whole mag chain (Square, add, sqrt,
                # reduce) runs within iteration j so the drain tail is short
                # and the single psB buffer frees before B(j+1) hits the PE
                aout = {0: stage_a(0)}
                m2s, sps = {}, {}
                for j in range(N_WC + 3):
                    if 0 <= j < N_WC:
                        xy = stage_copy(j, *aout.pop(j))
                    if j - 3 in sps:
                        stage_pool(j - 3, sps.pop(j - 3))
                    if j + 1 < N_WC:
                        aout[j + 1] = stage_a(j + 1)
                    if 0 <= j < N_WC:
                        m2s[j] = stage_b(j, xy)
                    if j - 1 in m2s:
                        sps[j - 1] = stage_mag2(j - 1, m2s.pop(j - 1))

                # ---- sigmoid(5x-1)^2 on pooled, store deferred to the end
                sg = outp.tile([128, 2 * WP], F32, tag="sg")
                nc.scalar.activation(
                    sg[0:96, :], pooled[0:96, :], AF.Sigmoid,
                    bias=bias_m1[0:96, :], scale=5.0,
                )
                ot = outp.tile([128, 2 * WP], F32, tag="ot")
                nc.gpsimd.tensor_mul(ot[0:96, :], sg[0:96, :], sg[0:96, :])
                stores.append((b, ot))

            for b, ot in stores:
                nc.sync.dma_start(y[b, 0, 0:96, :], ot[0:96, 0:WP])
                nc.sync.dma_start(y[b, 0, 96:HP, :], ot[0:40, WP : 2 * WP])

    split_multi_waits(nc)
    return nc


_NC = None
_CONSTS = None
TRACE = False
LAST_EXEC_NS = None


def kernel(**inputs):
    global _NC, _CONSTS, LAST_EXEC_NS
    left_rgb = np.ascontiguousarray(np.asarray(inputs["left_rgb"], dtype=np.float32))
    assert left_rgb.shape == (B_FULL, C, H, W)
    if _NC is None:
        _NC = build_module()
        _CONSTS = build_constants()
    band_a, band_b, p4 = _CONSTS
    in_maps = [
        {
            "x": np.ascontiguousarray(left_rgb[i * B_LOC : (i + 1) * B_LOC]),
            "bA": band_a,
            "bB": band_b,
            "p4": p4,
        }
        for i in range(N_CORES)
    ]
    res = run_bass_kernel_spmd(
        _NC, in_maps, core_ids=list(range(N_CORES)), trace=TRACE
    )
    LAST_EXEC_NS = res.exec_time_ns
    out = np.empty((B_FULL, 1, HP, WP), dtype=np.float32)
    for i in range(N_CORES):
        out[i * B_LOC : (i + 1) * B_LOC] = res.results[i]["y"]
    return out
